# revision 3
# baseline (speedup 1.0000x reference)
"""FAME-GCN Trainium2 kernel — bf16 adjacency variant.

Same structure as the fp32 baseline (row-sharded 625 rows/core, stripe-major
gathers + DVE merge + two matmul directions), but the adjacency stripes are
cast to bf16 on the host. This halves the HBM traffic (the memory-roofline
term), doubles DVE merge throughput, and keeps PE cost identical (PE streams
one column/cycle regardless of dtype). PSUM accumulation stays fp32.

Error budget: bf16 rounding of A contributes ~0.1-0.5% relative error against
a 2e-2 tolerance (quantization errors grow as sqrt(N) like the random-sign
signal, so the ratio is ~2^-9-level, not washed out).
"""

import sys

if "/opt/trn_rl_repo" not in sys.path:
    sys.path.insert(0, "/opt/trn_rl_repo")

import numpy as np
import ml_dtypes

import concourse.bacc as bacc
import concourse.mybir as mybir
from concourse.tile import TileContext
from concourse.bass_utils import run_bass_kernel_spmd

F32 = mybir.dt.float32
BF16 = mybir.dt.bfloat16
I16 = mybir.dt.int16
MUL = mybir.AluOpType.mult
ADD = mybir.AluOpType.add

NPBF = ml_dtypes.bfloat16

N = 5000
NP = 5120  # padded row length (bf16 row = 10240 B, multiple of 256)
NFEAT = 128
OUT = 16
K_A, K_AT = 3, 9
NCORES = 8
RS = N // NCORES  # 625 rows per core
STRIPE = 125
NSTRIPE = RS // STRIPE
CB = 512
NCB = (N + CB - 1) // CB

_CACHE = {}


def _c_blocks():
    return [(cb * CB, min(CB, N - cb * CB)) for cb in range(NCB)]


def _c_subs_all():
    subs = []
    j = 0
    while j * 128 < N:
        subs.append((j, min(128, N - j * 128)))
        j += 1
    return subs  # 40 chunks on the global 128-grid


def build():
    nc = bacc.Bacc(num_swdge_queues=4)

    a = nc.declare_dram_parameter("a", [K_A, RS, NP], BF16, isOutput=False)
    at = nc.declare_dram_parameter("at", [K_AT, RS, NP], BF16, isOutput=False)
    idxs = nc.declare_dram_parameter("idxs", [128, 8 * NSTRIPE], I16, isOutput=False)
    nchunks0 = (N + 127) // 128
    s3f_d = nc.declare_dram_parameter("s3f", [128, nchunks0 * OUT], BF16, isOutput=False)
    s1f_d = nc.declare_dram_parameter("s1f", [128, nchunks0 * OUT], BF16, isOutput=False)
    s3o_d = nc.declare_dram_parameter("s3o", [STRIPE, NSTRIPE * OUT], BF16, isOutput=False)
    s1o_d = nc.declare_dram_parameter("s1o", [STRIPE, NSTRIPE * OUT], BF16, isOutput=False)
    identb = nc.declare_dram_parameter("identb", [128, 128], BF16, isOutput=False)

    o1a = nc.declare_dram_parameter("o1a", [OUT, N], F32, isOutput=True)
    o1b = nc.declare_dram_parameter("o1b", [OUT, N], F32, isOutput=True)
    o2a = nc.declare_dram_parameter("o2a", [OUT, RS], F32, isOutput=True)
    o2b = nc.declare_dram_parameter("o2b", [OUT, RS], F32, isOutput=True)

    nchunks = (N + 127) // 128  # 40

    with TileContext(nc) as tc:
        with (
            tc.tile_pool(name="persist", bufs=1) as pp,
            tc.tile_pool(name="pt", bufs=2, space="PSUM") as ptp,
            tc.tile_pool(name="pd1", bufs=2, space="PSUM") as pd1p,
            tc.tile_pool(name="pd2", bufs=2, space="PSUM") as pd2p,
        ):
            # ---------------- persistent tiles ----------------
            idb_t = pp.tile([128, 128], BF16, tag="identb")
            nc.sync.dma_start(out=idb_t, in_=identb[:, :])
            ix = pp.tile([128, 8 * NSTRIPE], I16, tag="ix")
            nc.sync.dma_start(out=ix, in_=idxs[:, :])

            # S matrices (host-computed, packed layouts)
            s3f = pp.tile([128, nchunks * OUT], BF16, tag="s3f")
            s1f = pp.tile([128, nchunks * OUT], BF16, tag="s1f")
            s3o = pp.tile([STRIPE, NSTRIPE * OUT], BF16, tag="s3o")
            s1o = pp.tile([STRIPE, NSTRIPE * OUT], BF16, tag="s1o")
            nc.sync.dma_start(out=s3f, in_=s3f_d[:, :])
            nc.sync.dma_start(out=s1f, in_=s1f_d[:, :])
            nc.sync.dma_start(out=s3o, in_=s3o_d[:, :])
            nc.sync.dma_start(out=s1o, in_=s1o_d[:, :])

            o1sb_a = pp.tile([OUT, N], F32, tag="o1sb_a")
            o1sb_b = pp.tile([OUT, N], F32, tag="o1sb_b")
            acc2a = pp.tile([OUT, RS], F32, tag="acc2a")
            acc2b = pp.tile([OUT, RS], F32, tag="acc2b")

            # ---------------- main loop: stripe-major ----------------
            with (
                tc.tile_pool(name="raw", bufs=8) as rawp,
                tc.tile_pool(name="mrg", bufs=2) as mrgp,
                tc.tile_pool(name="ttp", bufs=4) as ttp,
            ):
                groups = (
                    ("a", s3o, s3f, acc2a, o1sb_a),
                    ("b", s1o, s1f, acc2b, o1sb_b),
                )
                for st in range(NSTRIPE):
                    ixs = ix[:, st * 8 : (st + 1) * 8]
                    r0 = st * STRIPE
                    # 10 relation stripes via SWDGE gathers (4 queues);
                    # k=10,11 ride the near-idle sync HWDGE ring instead
                    th = {}
                    for k in range(K_A + K_AT):
                        src = a[k, :, :] if k < K_A else at[k - K_A, :, :]
                        t = rawp.tile(
                            [128, 1, NP], BF16, tag="traw",
                            name=f"t_{st}_{k}",
                        )
                        if k >= 10:
                            nc.sync.dma_start(
                                out=t[:STRIPE, 0, :],
                                in_=src[r0 : r0 + STRIPE, :],
                            )
                        else:
                            nc.gpsimd.dma_gather(
                                t,
                                src[:, :],
                                ixs,
                                128,
                                128,
                                NP,
                                elem_step=NP,
                                queue_num=k % 4,
                            )
                        th[k] = t
                    # merge: host pre-scaled by w_k, so pure add-trees on DVE
                    # (tensor_tensor bf16 hits the 2x packed mode; the
                    # scalar_tensor_tensor FMA chain did not)
                    mrga = mrgp.tile([128, N], BF16, tag="mrga", name=f"mrga_{st}")
                    mrgb = mrgp.tile([128, N], BF16, tag="mrgb", name=f"mrgb_{st}")

                    def tk(k):
                        return th[k][:, 0, :N]

                    nc.vector.tensor_add(mrga[:, :], tk(0), tk(1))
                    nc.vector.tensor_add(mrga[:, :], mrga[:, :], tk(2))
                    nc.vector.tensor_add(mrgb[:, :], tk(3), tk(4))
                    for k in (5, 6, 7, 8, 9, 10, 11):
                        nc.vector.tensor_add(mrgb[:, :], mrgb[:, :], tk(k))

                    for gname, so, sf, acc2, o1sb in groups:
                        mrg = mrga if gname == "a" else mrgb
                        # dir1: o1sb[:, blk] (+)= S_own[st]^T @ mrg[:, blk]
                        for cb, (c0, cw) in enumerate(_c_blocks()):
                            pd1 = pd1p.tile(
                                [OUT, CB], F32, tag="pd1",
                                name=f"pd1_{st}_{gname}_{cb}",
                            )
                            nc.tensor.matmul(
                                pd1[:, :cw],
                                so[:, st * OUT : (st + 1) * OUT],
                                mrg[:STRIPE, c0 : c0 + cw],
                                start=True,
                                stop=True,
                            )
                            dst = o1sb[:, c0 : c0 + cw]
                            if st == 0:
                                nc.vector.tensor_copy(out=dst, in_=pd1[:, :cw])
                            else:
                                nc.vector.tensor_add(dst, dst, pd1[:, :cw])
                        # dir2: acc2[:, st] = sum_j S[c_j]^T @ (mrg[:, c_j])^T
                        pd2 = pd2p.tile(
                            [OUT, 128], F32, tag="pd2", name=f"pd2_{st}_{gname}"
                        )
                        allsubs = _c_subs_all()
                        for j, cjw in allsubs:
                            ptr = ptp.tile(
                                [128, 128], BF16, tag="pt",
                                name=f"pt_{st}_{gname}_{j}",
                            )
                            nc.tensor.transpose(
                                ptr[:cjw, :126],
                                mrg[:STRIPE, 128 * j : 128 * j + cjw],
                                idb_t[:STRIPE, :126],
                            )
                            strip = ttp.tile(
                                [128, 126], BF16, tag="tt",
                                name=f"tt_{st}_{gname}_{j}",
                            )
                            nc.scalar.copy(out=strip[:cjw, :], in_=ptr[:cjw, :126])
                            nc.tensor.matmul(
                                pd2[:, :126],
                                sf[:cjw, j * OUT : (j + 1) * OUT],
                                strip[:cjw, :],
                                start=(j == 0),
                                stop=(j == len(allsubs) - 1),
                            )
                        nc.vector.tensor_copy(
                            out=acc2[:, st * STRIPE : (st + 1) * STRIPE],
                            in_=pd2[:, :STRIPE],
                        )

            nc.sync.dma_start(out=o1a[:, :], in_=o1sb_a)
            nc.sync.dma_start(out=o1b[:, :], in_=o1sb_b)
            nc.sync.dma_start(out=o2a[:, :], in_=acc2a)
            nc.sync.dma_start(out=o2b[:, :], in_=acc2b)

    nc.compile()
    return nc


def _pack_sf(S):
    nchunks = (N + 127) // 128
    out = np.zeros((128, nchunks * OUT), dtype=NPBF)
    for t in range(nchunks):
        r0 = t * 128
        wt = min(128, N - r0)
        out[:wt, t * OUT : (t + 1) * OUT] = S[r0 : r0 + wt, :].astype(NPBF)
    return out


def _make_inputs(feature, A, A_t, w2, wb, W3, W1):
    eyeb = np.eye(128, dtype=NPBF)
    S3 = (feature @ W3).astype(np.float32)
    S1 = (feature @ W1).astype(np.float32)
    s3f = _pack_sf(S3)
    s1f = _pack_sf(S1)
    idxs = np.full((128, 8 * NSTRIPE), -1, dtype=np.int16)
    for st in range(NSTRIPE):
        for j in range(STRIPE):
            for rep in range(8):
                idxs[j % 16 + 16 * rep, st * 8 + j // 16] = STRIPE * st + j

    # pre-scale by merge weights during the bf16 cast (merge becomes add-only)
    apad = np.zeros((K_A, N, NP), dtype=NPBF)
    apad[:, :, :N] = (A * w2[:, None, None]).astype(NPBF)
    atpad = np.zeros((K_AT, N, NP), dtype=NPBF)
    atpad[:, :, :N] = (A_t * wb[:, None, None]).astype(NPBF)

    in_maps = []
    for p in range(NCORES):
        r0 = p * RS
        s3o = np.zeros((STRIPE, NSTRIPE * OUT), dtype=NPBF)
        s1o = np.zeros((STRIPE, NSTRIPE * OUT), dtype=NPBF)
        for u in range(NSTRIPE):
            rr = r0 + u * STRIPE
            s3o[:, u * OUT : (u + 1) * OUT] = S3[rr : rr + STRIPE, :].astype(NPBF)
            s1o[:, u * OUT : (u + 1) * OUT] = S1[rr : rr + STRIPE, :].astype(NPBF)
        in_maps.append(
            {
                "a": np.ascontiguousarray(apad[:, r0 : r0 + RS, :]),
                "at": np.ascontiguousarray(atpad[:, r0 : r0 + RS, :]),
                "idxs": idxs,
                "s3f": s3f,
                "s1f": s1f,
                "s3o": s3o,
                "s1o": s1o,
                "identb": eyeb,
            }
        )
    return in_maps


def kernel(feature, A, A_t, weight_b2, weight_b, W3, b3, W1, b1, **kw):
    feature = np.asarray(feature, dtype=np.float32)
    A = np.asarray(A, dtype=np.float32)
    A_t = np.asarray(A_t, dtype=np.float32)
    w2 = np.asarray(weight_b2, dtype=np.float32).reshape(K_A)
    wb = np.asarray(weight_b, dtype=np.float32).reshape(K_AT)
    W3 = np.asarray(W3, dtype=np.float32)
    W1 = np.asarray(W1, dtype=np.float32)
    b3 = np.asarray(b3, dtype=np.float32)
    b1 = np.asarray(b1, dtype=np.float32)

    if "nc" not in _CACHE:
        _CACHE["nc"] = build()
    nc = _CACHE["nc"]

    in_maps = _make_inputs(feature, A, A_t, w2, wb, W3, W1)
    _CACHE["in_maps"] = in_maps

    res = run_bass_kernel_spmd(nc, in_maps, core_ids=list(range(NCORES)))

    col_a = np.zeros((OUT, N), dtype=np.float32)
    col_b = np.zeros((OUT, N), dtype=np.float32)
    row_a = np.empty((OUT, N), dtype=np.float32)
    row_b = np.empty((OUT, N), dtype=np.float32)
    for p in range(NCORES):
        r = res.results[p]
        col_a += r["o1a"]
        col_b += r["o1b"]
        row_a[:, p * RS : (p + 1) * RS] = r["o2a"]
        row_b[:, p * RS : (p + 1) * RS] = r["o2b"]

    U1 = (col_a + row_a).T + b3
    U2 = (col_b + row_b).T + b1
    return np.concatenate([U1, U2], axis=1).astype(np.float32)
